# revision 21
# baseline (speedup 1.0000x reference)
"""AttnDecoderRNN single-step decode on 8 TRN2 NeuronCores.

Sharding: the vocab dimension of the output projection (out_w/out_b) is
split across the 8 cores (6400 rows each after padding 50257 -> 51200);
the tiny recurrent step (attention + comb + 4-layer GRU) is replicated on
every core. Each core computes its slice of the logits, the local
sum(exp(logits)), an 8-way AllGather combines the partial sums, and each
core writes log_softmax for its slice. The host only slices/re-lays-out
inputs and concatenates outputs.

Matmuls are row-form (weights stream as the moving operand — much faster
than fp32 stationary loads); the GRU's hidden-side products and all bias
terms are computed off the critical path.
"""

import numpy as np

HIDDEN = 256
NLAYERS = 4
MAXLEN = 256
VOCAB = 50257
NCORES = 8
P = 128
VP = 6400           # padded vocab rows per core
VPAD = VP * NCORES  # 51200
NEG = -1.0e30       # out_b padding: exp() underflows to 0

# compute dtype knobs: "f32" | "f32r" | "bf16"
WDT_NAME = "f32r"   # big vocab matvec (out_w, x)
GDT_NAME = "f32r"   # recurrent-chain weights (attn/comb/gru, their vectors)

_CACHE = {}


def _dt(name, mybir):
    return {"f32": mybir.dt.float32, "f32r": mybir.dt.float32r,
            "bf16": mybir.dt.bfloat16}[name]


# packB layout (single [1, *] row blob), float offsets
_OFF_ATTN_B = 0
_OFF_COMB_B = 256
_OFF_BIAS_GH = 512                     # per layer [brz(512) | bhn(256)] = 768
_OFF_BIN = _OFF_BIAS_GH + 4 * 768      # per layer 256
_OFF_HID_ROW = _OFF_BIN + 4 * 256      # per layer 256
_OFF_OUTB = _OFF_HID_ROW + 4 * 256     # 6400
_PACKB_LEN = _OFF_OUTB + VP

# v-tiles of the big matvec: 12 x 512 + 1 x 256
_VT = [(i * 512, 512) for i in range(12)] + [(12 * 512, 256)]


def _build(wdt_name, gdt_name):
    import concourse.mybir as mybir
    import concourse.tile as tile
    from concourse import bacc, bass
    from contextlib import ExitStack

    f32 = mybir.dt.float32
    i32 = mybir.dt.int32
    WDT = _dt(wdt_name, mybir)
    GDT = _dt(gdt_name, mybir)
    AF = mybir.ActivationFunctionType
    OP = mybir.AluOpType

    nc = bacc.Bacc(
        "TRN2",
        target_bir_lowering=False,
        debug=False,
        enable_asserts=False,
        num_devices=NCORES,
    )

    # ---------------- DRAM I/O ----------------
    token_d = nc.dram_tensor("token", [1, 1], i32, kind="ExternalInput")
    emb_d = nc.dram_tensor("emb", [VOCAB, HIDDEN], f32, kind="ExternalInput")
    pack1_d = nc.dram_tensor("pack1", [P, 1544], GDT, kind="ExternalInput")
    pack2_d = nc.dram_tensor("pack2", [P, 1024], GDT, kind="ExternalInput")
    packb_d = nc.dram_tensor("packb", [1, _PACKB_LEN], f32, kind="ExternalInput")
    gruw_d = nc.dram_tensor("gru_wt", [NLAYERS, 2, 2, P, 768], GDT, kind="ExternalInput")
    wt_d = nc.dram_tensor("wt", [2, P, VP], WDT, kind="ExternalInput")

    logp_d = nc.dram_tensor("logp", [1, VP], f32, kind="ExternalOutput")
    hidout_d = nc.dram_tensor("hidden_out", [NLAYERS, HIDDEN], f32, kind="ExternalOutput")
    attnout_d = nc.dram_tensor("attn_out", [1, MAXLEN], f32, kind="ExternalOutput")

    with tile.TileContext(nc) as tc, ExitStack() as ctx:
        wp = ctx.enter_context(tc.tile_pool(name="wp", bufs=1))
        sm = ctx.enter_context(tc.tile_pool(name="sm", bufs=1))
        ps = ctx.enter_context(tc.tile_pool(name="ps", bufs=1, space="PSUM"))
        dr = ctx.enter_context(tc.tile_pool(name="dr", bufs=1, space="DRAM"))

        # ---------------- constants ----------------
        ones_g = sm.tile([1, 1], f32, name="ones_g")
        nc.vector.memset(ones_g, 1.0)
        ones_col = sm.tile([P, 1], f32, name="ones_col")
        nc.vector.memset(ones_col, 1.0)
        zero_sb = sm.tile([1, 1], f32, name="zero_sb")
        nc.vector.memset(zero_sb, 0.0)
        one = ones_g[0:1, 0:1]

        # ---------------- small DMAs (first in HWDGE FIFO) ----------------
        idx_sb = sm.tile([2, 1], i32, name="idx_sb")
        nc.sync.dma_start(out=idx_sb[:, :], in_=token_d.ap().to_broadcast([2, 1]))

        packb_sb = sm.tile([1, _PACKB_LEN], f32, name="packb_sb")
        nc.scalar.dma_start(out=packb_sb[:, :], in_=packb_d.ap())

        pack1_sb = wp.tile([P, 1544], GDT, name="pack1_sb")
        nc.sync.dma_start(out=pack1_sb[:, :], in_=pack1_d.ap())
        pack2_sb = wp.tile([P, 1024], GDT, name="pack2_sb")
        nc.sync.dma_start(out=pack2_sb[:, :], in_=pack2_d.ap())

        attnw = pack1_sb[:, 0:1024].rearrange("p (k n) -> p k n", k=4)
        encw = pack1_sb[:, 1024:1536].rearrange("p (j n) -> p j n", j=2)
        hid_col = pack1_sb[:, 1536:1544].rearrange("p (l c) -> p l c", l=NLAYERS)
        combw = pack2_sb[:, 0:1024].rearrange("p (k n) -> p k n", k=4)

        def pb(off, n):
            return packb_sb[0:1, off:off + n]

        # AllGather warm-up doorbell: first on the gpsimd queue
        cc_in_w = dr.tile([1, 8], f32, name="cc_in_w")
        cc_out_w = dr.tile([NCORES, 8], f32, name="cc_out_w", addr_space="Shared")
        ws_sb = sm.tile([1, 8], f32, name="ws_sb")
        nc.vector.memset(ws_sb, 0.0)
        nc.gpsimd.dma_start(out=cc_in_w[:, :], in_=ws_sb[:, :])
        nc.gpsimd.collective_compute(
            "AllGather", OP.bypass,
            replica_groups=[list(range(NCORES))],
            ins=[cc_in_w.opt()], outs=[cc_out_w.opt()],
        )

        # embedding row gather (SWDGE path)
        emb_row = sm.tile([2, HIDDEN], f32, name="emb_row")
        nc.gpsimd.indirect_dma_start(
            out=emb_row[:, :],
            out_offset=None,
            in_=emb_d.ap(),
            in_offset=bass.IndirectOffsetOnAxis(ap=idx_sb[:, 0:1], axis=0),
        )

        # ---------------- bulk weight DMAs (GRU first, then vocab shard) -------
        gw = []
        for l in range(NLAYERS):
            g = wp.tile([P, 2, 2, 768], GDT, name=f"gw{l}")
            nc.sync.dma_start(
                out=g[:, :, :, :],
                in_=gruw_d.ap()[l].rearrange("w k p m -> p w k m"),
            )
            gw.append(g)

        NWT = 2  # DMA chunks per contraction half
        wt_sb = []  # wt_sb[kc] : [128, VP]
        for kc in range(2):
            t = wp.tile([P, NWT, VP // NWT], WDT, name=f"wt{kc}")
            wt_sb.append(t)
        for ck in range(NWT):
            for kc in range(2):
                s = ck * (VP // NWT)
                nc.sync.dma_start(
                    out=wt_sb[kc][:, ck, :],
                    in_=wt_d.ap()[kc, :, s:s + VP // NWT],
                )

        def wtv(kc, v0, n):  # [128, n] slice of the kc-half weight row block
            flat = wt_sb[kc][:, :, :].rearrange("p a b -> p (a b)")
            return flat[:, v0:v0 + n]

        # ---------------- ACT table pre-warm + AllGather warm-up ----------------
        # The first collective on a freshly loaded NEFF costs ~35-45us of ncfw
        # processing; fire a dummy AllGather as early as possible so that cost
        # (and the cross-core launch skew) is absorbed under the DMA/compute
        # phase, leaving the real AllGather cheap.
        warm = sm.tile([1, 4], f32, name="warm")
        nc.scalar.activation(out=warm[0:1, 0:1], in_=zero_sb[0:1, 0:1], func=AF.Exp)
        nc.scalar.activation(out=warm[0:1, 1:2], in_=zero_sb[0:1, 0:1], func=AF.Sigmoid)

        wg_sb = sm.tile([NCORES, 8], f32, name="wg_sb")
        nc.gpsimd.dma_start(out=wg_sb[:, :], in_=cc_out_w[:, :])

        # ---------------- embedded -> column form [128, 2] ----------------------
        ps_e = ps.tile([P, 2], f32, name="ps_e", tag="tp", bufs=1)
        nc.tensor.transpose(out=ps_e[:, 0:1], in_=emb_row[0:1, 0:P], identity=ones_col[0:1, 0:1])
        nc.tensor.transpose(out=ps_e[:, 1:2], in_=emb_row[0:1, P:2 * P], identity=ones_col[0:1, 0:1])
        e_col = sm.tile([P, 2], GDT, name="e_col")
        nc.vector.tensor_copy(out=e_col[:, :], in_=ps_e[:, :])

        # ---------------- attention: softmax(attn_in @ attn_w.T + b) ------------
        ps_al = ps.tile([1, MAXLEN], f32, name="ps_al", tag="rz", bufs=2)
        nc.tensor.matmul(ps_al[0:1, :], lhsT=one, rhs=pb(_OFF_ATTN_B, 256), start=True, stop=False)
        for kc in (2, 3):  # hidden-state side: ready before the embedding gather
            nc.tensor.matmul(ps_al[0:1, :], lhsT=hid_col[:, 0, kc - 2:kc - 1],
                             rhs=attnw[:, kc, :], start=False, stop=False)
        for kc in (0, 1):
            nc.tensor.matmul(ps_al[0:1, :], lhsT=e_col[:, kc:kc + 1],
                             rhs=attnw[:, kc, :], start=False, stop=(kc == 1))

        aw_e = sm.tile([1, MAXLEN], f32, name="aw_e")
        aw_s = sm.tile([1, 1], f32, name="aw_s")
        nc.scalar.activation(out=aw_e[:, :], in_=ps_al[0:1, :], func=AF.Exp, accum_out=aw_s[:, :])
        rinv = sm.tile([1, 1], f32, name="rinv")
        nc.vector.reciprocal(out=rinv[:, :], in_=aw_s[:, :])
        nc.vector.tensor_scalar_mul(out=aw_e[:, :], in0=aw_e[:, :], scalar1=rinv[0:1, 0:1])
        nc.scalar.dma_start(out=attnout_d.ap(), in_=aw_e[:, :])

        ps_awc = ps.tile([P, 2], f32, name="ps_awc", tag="tp", bufs=1)
        nc.tensor.transpose(out=ps_awc[:, 0:1], in_=aw_e[0:1, 0:P], identity=ones_col[0:1, 0:1])
        nc.tensor.transpose(out=ps_awc[:, 1:2], in_=aw_e[0:1, P:2 * P], identity=ones_col[0:1, 0:1])
        awc_sb = sm.tile([P, 2], f32, name="awc_sb")
        nc.vector.tensor_copy(out=awc_sb[:, :], in_=ps_awc[:, :])

        # attn_applied = softmax @ encoder_outputs (column form, N=1 -> f32)
        encw_f = encw if GDT != mybir.dt.float32r else encw.bitcast(f32)
        ps_app = ps.tile([P, 2], f32, name="ps_app", tag="tp", bufs=1)
        for hc in range(2):
            for jc in range(2):
                nc.tensor.matmul(ps_app[:, hc:hc + 1],
                                 lhsT=encw_f[:, jc, hc * P:(hc + 1) * P],
                                 rhs=awc_sb[:, jc:jc + 1],
                                 start=(jc == 0), stop=(jc == 1))
        app_sb = sm.tile([P, 2], GDT, name="app_sb")
        nc.vector.tensor_copy(out=app_sb[:, :], in_=ps_app[:, :])

        # x = relu(comb_in @ comb_w.T + comb_b)  (row form; relu on DVE)
        ps_x = ps.tile([1, HIDDEN], f32, name="ps_x", tag="in", bufs=2)
        nc.tensor.matmul(ps_x[0:1, :], lhsT=one, rhs=pb(_OFF_COMB_B, 256), start=True, stop=False)
        for kc in (0, 1):
            nc.tensor.matmul(ps_x[0:1, :], lhsT=e_col[:, kc:kc + 1],
                             rhs=combw[:, kc, :], start=False, stop=False)
        for kc in (2, 3):
            nc.tensor.matmul(ps_x[0:1, :], lhsT=app_sb[:, kc - 2:kc - 1],
                             rhs=combw[:, kc, :], start=False, stop=(kc == 3))
        x_row = sm.tile([1, HIDDEN], f32, name="x_row")
        nc.vector.tensor_scalar_max(out=x_row[:, :], in0=ps_x[0:1, :], scalar1=0.0)

        def to_col(row, name, dt_):
            pst = ps.tile([P, 2], f32, name=f"ps_{name}", tag="tp", bufs=1)
            nc.tensor.transpose(out=pst[:, 0:1], in_=row[0:1, 0:P], identity=ones_col[0:1, 0:1])
            nc.tensor.transpose(out=pst[:, 1:2], in_=row[0:1, P:2 * P], identity=ones_col[0:1, 0:1])
            col = sm.tile([P, 2], dt_, name=name, tag="xcol", bufs=3)
            nc.vector.tensor_copy(out=col[:, :], in_=pst[:, :])
            return col

        x_col = to_col(x_row, "x_col0", GDT)

        # ---------------- GRU: hidden-side products + biases, accumulated --------
        # directly into each layer's psum groups, off the critical path.
        ps_rz_l, ps_hn_l, ps_in_l = [], [], []
        for l in range(NLAYERS):
            ps_rz = ps.tile([1, 512], f32, name=f"ps_rz{l}", tag="rz", bufs=2)
            ps_hn = ps.tile([1, 256], f32, name=f"ps_hn{l}", tag="hn", bufs=1)
            ps_in = ps.tile([1, 256], f32, name=f"ps_in{l}", tag="in", bufs=2)
            ob = _OFF_BIAS_GH + l * 768
            nc.tensor.matmul(ps_rz[0:1, :], lhsT=one, rhs=pb(ob, 512), start=True, stop=False)
            for kc in range(2):
                nc.tensor.matmul(ps_rz[0:1, :], lhsT=hid_col[:, l, kc:kc + 1],
                                 rhs=gw[l][:, 1, kc, 0:512], start=False, stop=False)
            nc.tensor.matmul(ps_hn[0:1, :], lhsT=one, rhs=pb(ob + 512, 256), start=True, stop=False)
            for kc in range(2):
                nc.tensor.matmul(ps_hn[0:1, :], lhsT=hid_col[:, l, kc:kc + 1],
                                 rhs=gw[l][:, 1, kc, 512:768], start=False, stop=(kc == 1))
            nc.tensor.matmul(ps_in[0:1, :], lhsT=one, rhs=pb(_OFF_BIN + l * 256, 256),
                             start=True, stop=False)
            ps_rz_l.append(ps_rz); ps_hn_l.append(ps_hn); ps_in_l.append(ps_in)

        # ---------------- GRU chain ----------------------------------------------
        for l in range(NLAYERS):
            ps_rz, ps_hn, ps_in = ps_rz_l[l], ps_hn_l[l], ps_in_l[l]
            for kc in range(2):
                nc.tensor.matmul(ps_rz[0:1, :], lhsT=x_col[:, kc:kc + 1],
                                 rhs=gw[l][:, 0, kc, 0:512], start=False, stop=(kc == 1))
                nc.tensor.matmul(ps_in[0:1, :], lhsT=x_col[:, kc:kc + 1],
                                 rhs=gw[l][:, 0, kc, 512:768], start=False, stop=(kc == 1))

            rz = sm.tile([1, 512], f32, name="rz", tag="rz_sb", bufs=1)
            nc.scalar.activation(out=rz[:, :], in_=ps_rz[0:1, :], func=AF.Sigmoid)
            rhn = sm.tile([1, 256], f32, name="rhn", tag="gtmp", bufs=3)
            nc.vector.tensor_mul(out=rhn[:, :], in0=rz[0:1, 0:256], in1=ps_hn[0:1, :])
            npre = sm.tile([1, 256], f32, name="npre", tag="gtmp", bufs=3)
            nc.vector.tensor_add(out=npre[:, :], in0=ps_in[0:1, :], in1=rhn[:, :])
            # tanh(v) = 2*sigmoid(2v) - 1  (keeps the sigmoid ACT table resident)
            sg2 = sm.tile([1, 256], f32, name="sg2", tag="gtmp", bufs=3)
            nc.scalar.activation(out=sg2[:, :], in_=npre[:, :], func=AF.Sigmoid, scale=2.0)
            n_sb = sm.tile([1, 256], f32, name="n_sb", tag="gtmp", bufs=3)
            nc.vector.tensor_scalar(out=n_sb[:, :], in0=sg2[:, :], scalar1=2.0, scalar2=1.0,
                                    op0=OP.mult, op1=OP.subtract)
            d_sb = sm.tile([1, 256], f32, name="d_sb", tag="gtmp", bufs=3)
            nc.vector.tensor_sub(out=d_sb[:, :], in0=pb(_OFF_HID_ROW + l * 256, 256),
                                 in1=n_sb[:, :])
            zd_sb = sm.tile([1, 256], f32, name="zd_sb", tag="gtmp", bufs=3)
            nc.vector.tensor_mul(out=zd_sb[:, :], in0=rz[0:1, 256:512], in1=d_sb[:, :])
            h_row = sm.tile([1, 256], f32, name="h_row", tag="h_row", bufs=2)
            nc.vector.tensor_add(out=h_row[:, :], in0=n_sb[:, :], in1=zd_sb[:, :])
            nc.scalar.dma_start(out=hidout_d.ap()[l:l + 1, :], in_=h_row[:, :])
            x_col = to_col(h_row, f"x_col{l + 1}", WDT if l == NLAYERS - 1 else GDT)

        # ---------------- logits = x @ out_w.T + out_b  (vocab shard) ------------
        lg_sb = sm.tile([1, VP], f32, name="lg_sb")
        sacc = sm.tile([1, 8], f32, name="sacc")
        xv = x_col[:, :]
        for i, (v0, n) in enumerate(_VT):
            ps_row = ps.tile([1, 512], f32, name=f"ps_row{i}", tag="row", bufs=2)
            nc.tensor.matmul(ps_row[0:1, 0:n], lhsT=xv[:, 0:1], rhs=wtv(0, v0, n),
                             start=True, stop=False)
            nc.tensor.matmul(ps_row[0:1, 0:n], lhsT=xv[:, 1:2], rhs=wtv(1, v0, n),
                             start=False, stop=True)
            nc.vector.tensor_add(out=lg_sb[0:1, v0:v0 + n], in0=ps_row[0:1, 0:n],
                                 in1=pb(_OFF_OUTB + v0, n))

        # sum(exp(logits)) in 4 coarse slices on ACT
        dummy = sm.tile([1, 1600], f32, name="dummy", tag="dummy", bufs=2)
        for s in range(4):
            nc.scalar.activation(out=dummy[0:1, :], in_=lg_sb[0:1, s * 1600:(s + 1) * 1600],
                                 func=AF.Exp, accum_out=sacc[0:1, s:s + 1])

        s_sb = sm.tile([1, 8], f32, name="s_sb")
        nc.vector.memset(s_sb, 0.0)
        nc.vector.tensor_reduce(out=s_sb[0:1, 0:1], in_=sacc[0:1, 0:4],
                                axis=mybir.AxisListType.X, op=OP.add)

        # ---------------- 8-way AllGather of the partial sums ---------------------
        cc_in = dr.tile([1, 8], f32, name="cc_in")
        cc_out = dr.tile([NCORES, 8], f32, name="cc_out", addr_space="Shared")
        nc.gpsimd.dma_start(out=cc_in[:, :], in_=s_sb[:, :])
        nc.gpsimd.collective_compute(
            "AllGather", OP.bypass,
            replica_groups=[list(range(NCORES))],
            ins=[cc_in.opt()], outs=[cc_out.opt()],
        )
        sg_sb = sm.tile([NCORES, 8], f32, name="sg_sb")
        nc.gpsimd.dma_start(out=sg_sb[:, :], in_=cc_out[:, :])

        ps_sg = ps.tile([1, 1], f32, name="ps_sg", tag="tp", bufs=1)
        nc.tensor.matmul(ps_sg[0:1, 0:1], lhsT=sg_sb[:, 0:1], rhs=ones_col[0:NCORES, 0:1],
                         start=True, stop=True)
        ln_sb = sm.tile([1, 1], f32, name="ln_sb")
        nc.scalar.activation(out=ln_sb[:, :], in_=ps_sg[0:1, 0:1], func=AF.Ln)
        negln = sm.tile([1, 1], f32, name="negln")
        nc.vector.tensor_scalar_mul(out=negln[:, :], in0=ln_sb[:, :], scalar1=-1.0)

        # logp = logits - log(sum_exp), in place; alternate DVE/ACT slices
        for i in range(8):
            v0, n = i * 800, 800
            if i % 2 == 0:
                nc.vector.tensor_scalar_sub(out=lg_sb[0:1, v0:v0 + n],
                                            in0=lg_sb[0:1, v0:v0 + n],
                                            scalar1=ln_sb[0:1, 0:1])
            else:
                nc.scalar.activation(out=lg_sb[0:1, v0:v0 + n],
                                     in_=lg_sb[0:1, v0:v0 + n],
                                     func=AF.Identity, bias=negln[0:1, 0:1])
        nc.sync.dma_start(out=logp_d.ap(), in_=lg_sb[:, :])

    nc.compile()
    return nc


def _get_nc():
    key = (WDT_NAME, GDT_NAME)
    if key not in _CACHE:
        _CACHE[key] = _build(*key)
    return _CACHE[key]


def _np_dt(name):
    if name == "bf16":
        import ml_dtypes
        return np.dtype(ml_dtypes.bfloat16)
    return np.float32


def _prep_in_maps(inputs):
    f = lambda k: np.ascontiguousarray(np.asarray(inputs[k], dtype=np.float32))
    gnp = _np_dt(GDT_NAME)
    wnp = _np_dt(WDT_NAME)

    token = np.asarray(inputs["token"]).astype(np.int32).reshape(1, 1)
    emb = f("emb")

    hidden = f("hidden").reshape(NLAYERS, HIDDEN)
    hid_colmat = hidden.reshape(NLAYERS, 2, P).transpose(2, 0, 1).reshape(P, NLAYERS * 2)

    pack1 = np.concatenate([
        f("attn_w").T.reshape(4, P, MAXLEN).transpose(1, 0, 2).reshape(P, 1024),
        f("encoder_outputs").reshape(2, P, HIDDEN).transpose(1, 0, 2).reshape(P, 512),
        hid_colmat,
    ], axis=1).astype(gnp)
    pack2 = f("comb_w").T.reshape(4, P, HIDDEN).transpose(1, 0, 2).reshape(P, 1024).astype(gnp)
    assert pack1.shape == (P, 1544)

    bih, bhh = f("gru_bih"), f("gru_bhh")
    b_pad = np.full((VPAD,), NEG, np.float32)
    b_pad[:VOCAB] = f("out_b")
    packb = np.zeros((1, _PACKB_LEN), np.float32)
    packb[0, _OFF_ATTN_B:_OFF_ATTN_B + 256] = f("attn_b")
    packb[0, _OFF_COMB_B:_OFF_COMB_B + 256] = f("comb_b")
    for l in range(NLAYERS):
        o = _OFF_BIAS_GH + l * 768
        packb[0, o:o + 512] = bih[l, :512] + bhh[l, :512]   # brz (merged)
        packb[0, o + 512:o + 768] = bhh[l, 512:]            # bhn
        packb[0, _OFF_BIN + l * 256:_OFF_BIN + (l + 1) * 256] = bih[l, 512:]
        packb[0, _OFF_HID_ROW + l * 256:_OFF_HID_ROW + (l + 1) * 256] = hidden[l]

    wih, whh = f("gru_wih"), f("gru_whh")
    gru_wt = np.ascontiguousarray(
        np.stack([
            np.stack([wih[l].T.reshape(2, P, 768), whh[l].T.reshape(2, P, 768)])
            for l in range(NLAYERS)
        ])
    ).astype(gnp)  # [4, 2, 2, 128, 768]

    w_pad = np.zeros((VPAD, HIDDEN), np.float32)
    w_pad[:VOCAB] = f("out_w")

    common = dict(token=token, emb=emb, pack1=pack1, pack2=pack2, packb=packb, gru_wt=gru_wt)
    in_maps = []
    for k in range(NCORES):
        rows = slice(k * VP, (k + 1) * VP)
        wt = np.ascontiguousarray(w_pad[rows].T.reshape(2, P, VP)).astype(wnp)
        pb = packb.copy()
        pb[0, _OFF_OUTB:] = b_pad[rows]
        in_maps.append(dict(common, wt=wt, packb=pb))
    return in_maps


def _ensure_ntff_hook():
    """The agent image's ``antenv`` lacks ``axon_hooks``; shim it so
    run_bass_kernel_spmd(trace=True) can capture NTFF profiles via the
    libaxon_pjrt.so C ABI (same mechanism as trn_agent_boot)."""
    import sys
    import types
    import contextlib
    import ctypes

    try:
        from antenv.axon_hooks import get_axon_ntff_profile_hook  # noqa: F401
        return
    except ImportError:
        pass

    mod = types.ModuleType("antenv.axon_hooks")
    _state = {"hook": None}
    mod.set_axon_ntff_profile_hook = lambda h: _state.__setitem__("hook", h)
    mod.get_axon_ntff_profile_hook = lambda: _state["hook"]
    sys.modules["antenv.axon_hooks"] = mod

    so_path = "/opt/axon/libaxon_pjrt.so"
    try:
        lib = ctypes.CDLL(so_path)
    except OSError:
        return
    if not hasattr(lib, "axon_start_nrt_profile"):
        return
    lib.axon_start_nrt_profile.argtypes = [ctypes.POINTER(ctypes.c_int64), ctypes.c_size_t]
    lib.axon_start_nrt_profile.restype = ctypes.c_int64
    lib.axon_stop_nrt_profile.argtypes = [ctypes.c_char_p]
    lib.axon_stop_nrt_profile.restype = ctypes.c_int64

    @contextlib.contextmanager
    def _hook(output_dir, device_ids):
        import jax

        jax.devices()
        if device_ids:
            ids = (ctypes.c_int64 * len(device_ids))(*device_ids)
            rc = lib.axon_start_nrt_profile(ids, len(device_ids))
        else:
            rc = lib.axon_start_nrt_profile(None, 0)
        if rc != 0:
            raise RuntimeError(f"axon_start_nrt_profile rc={rc}")
        try:
            yield
        finally:
            n = lib.axon_stop_nrt_profile(str(output_dir).encode())
            print(f"ntff profile: {n} file(s) written to {output_dir}")

    mod.set_axon_ntff_profile_hook(_hook)


def run(inputs, trace=False, trace_cores=None):
    from concourse.bass_utils import run_bass_kernel_spmd

    if trace:
        _ensure_ntff_hook()
    nc = _get_nc()
    in_maps = _prep_in_maps(inputs)
    res = run_bass_kernel_spmd(nc, in_maps, core_ids=list(range(NCORES)), trace=trace,
                               trace_cores=trace_cores)
    outs = res.results
    logp = np.concatenate(
        [outs[k]["logp"][0] for k in range(NCORES)]
    )[:VOCAB].reshape(1, VOCAB).astype(np.float32)
    hidden_new = np.ascontiguousarray(outs[0]["hidden_out"].reshape(NLAYERS, 1, HIDDEN))
    attn_weights = np.ascontiguousarray(outs[0]["attn_out"].reshape(1, MAXLEN))
    return (logp, hidden_new, attn_weights), res


def kernel(**inputs):
    (logp, hidden_new, attn_weights), _ = run(inputs, trace=False)
    return logp, hidden_new, attn_weights


# revision 22
# speedup vs baseline: 1.3760x; 1.3760x over previous
"""AttnDecoderRNN single-step decode on 8 TRN2 NeuronCores.

Sharding: the vocab dimension of the output projection (out_w/out_b) is
split across the 8 cores (6400 rows each after padding 50257 -> 51200);
the tiny recurrent step (attention + comb + 4-layer GRU) is replicated on
every core. Each core computes its slice of the logits, the local
sum(exp(logits)), an 8-way AllGather combines the partial sums, and each
core writes log_softmax for its slice. The host only slices/re-lays-out
inputs and concatenates outputs.

Matmuls are row-form (weights stream as the moving operand — much faster
than fp32 stationary loads); the GRU's hidden-side products and all bias
terms are computed off the critical path.
"""

import numpy as np

HIDDEN = 256
NLAYERS = 4
MAXLEN = 256
VOCAB = 50257
NCORES = 8
P = 128
VP = 6400           # padded vocab rows per core
VPAD = VP * NCORES  # 51200
NEG = -1.0e30       # out_b padding: exp() underflows to 0

# compute dtype knobs: "f32" | "f32r" | "bf16"
WDT_NAME = "f32r"   # big vocab matvec (out_w, x)
GDT_NAME = "f32r"   # recurrent-chain weights (attn/comb/gru, their vectors)

_CACHE = {}


def _dt(name, mybir):
    return {"f32": mybir.dt.float32, "f32r": mybir.dt.float32r,
            "bf16": mybir.dt.bfloat16}[name]


# packB layout (single [1, *] row blob), float offsets
_OFF_ATTN_B = 0
_OFF_COMB_B = 256
_OFF_BIAS_GH = 512                     # per layer [brz(512) | bhn(256)] = 768
_OFF_BIN = _OFF_BIAS_GH + 4 * 768      # per layer 256
_OFF_HID_ROW = _OFF_BIN + 4 * 256      # per layer 256
_OFF_OUTB = _OFF_HID_ROW + 4 * 256     # 6400
_PACKB_LEN = _OFF_OUTB + VP

# v-tiles of the big matvec: 12 x 512 + 1 x 256
_VT = [(i * 512, 512) for i in range(12)] + [(12 * 512, 256)]


def _build(wdt_name, gdt_name):
    import concourse.mybir as mybir
    import concourse.tile as tile
    from concourse import bacc, bass
    from contextlib import ExitStack

    f32 = mybir.dt.float32
    i32 = mybir.dt.int32
    WDT = _dt(wdt_name, mybir)
    GDT = _dt(gdt_name, mybir)
    AF = mybir.ActivationFunctionType
    OP = mybir.AluOpType

    nc = bacc.Bacc(
        "TRN2",
        target_bir_lowering=False,
        debug=False,
        enable_asserts=False,
        num_devices=NCORES,
    )

    # ---------------- DRAM I/O ----------------
    token_d = nc.dram_tensor("token", [1, 1], i32, kind="ExternalInput")
    emb_d = nc.dram_tensor("emb", [VOCAB, HIDDEN], f32, kind="ExternalInput")
    pack1_d = nc.dram_tensor("pack1", [P, 1544], GDT, kind="ExternalInput")
    pack2_d = nc.dram_tensor("pack2", [P, 1024], GDT, kind="ExternalInput")
    packb_d = nc.dram_tensor("packb", [1, _PACKB_LEN], f32, kind="ExternalInput")
    gruw_d = nc.dram_tensor("gru_wt", [NLAYERS, 2, 2, P, 768], GDT, kind="ExternalInput")
    wt_d = nc.dram_tensor("wt", [2, P, VP], WDT, kind="ExternalInput")

    logp_d = nc.dram_tensor("logp", [1, VP], f32, kind="ExternalOutput")
    hidout_d = nc.dram_tensor("hidden_out", [NLAYERS, HIDDEN], f32, kind="ExternalOutput")
    attnout_d = nc.dram_tensor("attn_out", [1, MAXLEN], f32, kind="ExternalOutput")

    with tile.TileContext(nc) as tc, ExitStack() as ctx:
        wp = ctx.enter_context(tc.tile_pool(name="wp", bufs=1))
        sm = ctx.enter_context(tc.tile_pool(name="sm", bufs=1))
        ps = ctx.enter_context(tc.tile_pool(name="ps", bufs=1, space="PSUM"))
        dr = ctx.enter_context(tc.tile_pool(name="dr", bufs=1, space="DRAM"))

        # ---------------- constants ----------------
        ones_g = sm.tile([1, 1], f32, name="ones_g")
        nc.vector.memset(ones_g, 1.0)
        ones_col = sm.tile([P, 1], f32, name="ones_col")
        nc.vector.memset(ones_col, 1.0)
        zero_sb = sm.tile([1, 1], f32, name="zero_sb")
        nc.vector.memset(zero_sb, 0.0)
        one = ones_g[0:1, 0:1]

        # ---------------- small DMAs (first in HWDGE FIFO) ----------------
        idx_sb = sm.tile([2, 1], i32, name="idx_sb")
        nc.sync.dma_start(out=idx_sb[:, :], in_=token_d.ap().to_broadcast([2, 1]))

        packb_sb = sm.tile([1, _PACKB_LEN], f32, name="packb_sb")
        nc.scalar.dma_start(out=packb_sb[:, :], in_=packb_d.ap())

        pack1_sb = wp.tile([P, 1544], GDT, name="pack1_sb")
        nc.sync.dma_start(out=pack1_sb[:, :], in_=pack1_d.ap())
        pack2_sb = wp.tile([P, 1024], GDT, name="pack2_sb")
        nc.sync.dma_start(out=pack2_sb[:, :], in_=pack2_d.ap())

        attnw = pack1_sb[:, 0:1024].rearrange("p (k n) -> p k n", k=4)
        encw = pack1_sb[:, 1024:1536].rearrange("p (j n) -> p j n", j=2)
        hid_col = pack1_sb[:, 1536:1544].rearrange("p (l c) -> p l c", l=NLAYERS)
        combw = pack2_sb[:, 0:1024].rearrange("p (k n) -> p k n", k=4)

        def pb(off, n):
            return packb_sb[0:1, off:off + n]

        # AllGather warm-up doorbell: first on the gpsimd queue
        cc_in_w = dr.tile([1, 8], f32, name="cc_in_w")
        cc_out_w = dr.tile([NCORES, 8], f32, name="cc_out_w", addr_space="Shared")
        ws_sb = sm.tile([1, 8], f32, name="ws_sb")
        nc.vector.memset(ws_sb, 0.0)
        nc.gpsimd.dma_start(out=cc_in_w[:, :], in_=ws_sb[:, :])
        nc.gpsimd.collective_compute(
            "AllGather", OP.bypass,
            replica_groups=[list(range(NCORES))],
            ins=[cc_in_w.opt()], outs=[cc_out_w.opt()],
        )

        # embedding row gather (SWDGE path)
        emb_row = sm.tile([2, HIDDEN], f32, name="emb_row")
        nc.gpsimd.indirect_dma_start(
            out=emb_row[:, :],
            out_offset=None,
            in_=emb_d.ap(),
            in_offset=bass.IndirectOffsetOnAxis(ap=idx_sb[:, 0:1], axis=0),
        )

        # ---------------- bulk weight DMAs (GRU first, then vocab shard) -------
        gw = []
        for l in range(NLAYERS):
            g = wp.tile([P, 2, 2, 768], GDT, name=f"gw{l}")
            nc.sync.dma_start(
                out=g[:, :, :, :],
                in_=gruw_d.ap()[l].rearrange("w k p m -> p w k m"),
            )
            gw.append(g)

        NWT = 2  # DMA chunks per contraction half
        wt_sb = []  # wt_sb[kc] : [128, VP]
        for kc in range(2):
            t = wp.tile([P, NWT, VP // NWT], WDT, name=f"wt{kc}")
            wt_sb.append(t)
        for ck in range(NWT):
            for kc in range(2):
                s = ck * (VP // NWT)
                nc.sync.dma_start(
                    out=wt_sb[kc][:, ck, :],
                    in_=wt_d.ap()[kc, :, s:s + VP // NWT],
                )

        def wtv(kc, v0, n):  # [128, n] slice of the kc-half weight row block
            flat = wt_sb[kc][:, :, :].rearrange("p a b -> p (a b)")
            return flat[:, v0:v0 + n]

        # ---------------- ACT table pre-warm + AllGather warm-up ----------------
        # The first collective on a freshly loaded NEFF costs ~35-45us of ncfw
        # processing; fire a dummy AllGather as early as possible so that cost
        # (and the cross-core launch skew) is absorbed under the DMA/compute
        # phase, leaving the real AllGather cheap.
        warm = sm.tile([1, 4], f32, name="warm")
        nc.scalar.activation(out=warm[0:1, 0:1], in_=zero_sb[0:1, 0:1], func=AF.Exp)
        nc.scalar.activation(out=warm[0:1, 1:2], in_=zero_sb[0:1, 0:1], func=AF.Sigmoid)
        nc.scalar.activation(out=warm[0:1, 2:3], in_=ones_col[0:1, 0:1], func=AF.Ln)

        wg_sb = sm.tile([NCORES, 8], f32, name="wg_sb")
        nc.gpsimd.dma_start(out=wg_sb[:, :], in_=cc_out_w[:, :])

        # ---------------- embedded -> column form [128, 2] ----------------------
        ps_e = ps.tile([P, 2], f32, name="ps_e", tag="tp", bufs=1)
        nc.tensor.transpose(out=ps_e[:, 0:1], in_=emb_row[0:1, 0:P], identity=ones_col[0:1, 0:1])
        nc.tensor.transpose(out=ps_e[:, 1:2], in_=emb_row[0:1, P:2 * P], identity=ones_col[0:1, 0:1])
        e_col = sm.tile([P, 2], GDT, name="e_col")
        nc.vector.tensor_copy(out=e_col[:, :], in_=ps_e[:, :])

        # ---------------- attention: softmax(attn_in @ attn_w.T + b) ------------
        ps_al = ps.tile([1, MAXLEN], f32, name="ps_al", tag="rz", bufs=2)
        nc.tensor.matmul(ps_al[0:1, :], lhsT=one, rhs=pb(_OFF_ATTN_B, 256), start=True, stop=False)
        for kc in (2, 3):  # hidden-state side: ready before the embedding gather
            nc.tensor.matmul(ps_al[0:1, :], lhsT=hid_col[:, 0, kc - 2:kc - 1],
                             rhs=attnw[:, kc, :], start=False, stop=False)
        for kc in (0, 1):
            nc.tensor.matmul(ps_al[0:1, :], lhsT=e_col[:, kc:kc + 1],
                             rhs=attnw[:, kc, :], start=False, stop=(kc == 1))

        aw_e = sm.tile([1, MAXLEN], f32, name="aw_e")
        aw_s = sm.tile([1, 1], f32, name="aw_s")
        nc.scalar.activation(out=aw_e[:, :], in_=ps_al[0:1, :], func=AF.Exp, accum_out=aw_s[:, :])
        rinv = sm.tile([1, 1], f32, name="rinv")
        nc.vector.reciprocal(out=rinv[:, :], in_=aw_s[:, :])
        nc.vector.tensor_scalar_mul(out=aw_e[:, :], in0=aw_e[:, :], scalar1=rinv[0:1, 0:1])
        nc.scalar.dma_start(out=attnout_d.ap(), in_=aw_e[:, :])

        ps_awc = ps.tile([P, 2], f32, name="ps_awc", tag="tp", bufs=1)
        nc.tensor.transpose(out=ps_awc[:, 0:1], in_=aw_e[0:1, 0:P], identity=ones_col[0:1, 0:1])
        nc.tensor.transpose(out=ps_awc[:, 1:2], in_=aw_e[0:1, P:2 * P], identity=ones_col[0:1, 0:1])
        awc_sb = sm.tile([P, 2], f32, name="awc_sb")
        nc.vector.tensor_copy(out=awc_sb[:, :], in_=ps_awc[:, :])

        # attn_applied = softmax @ encoder_outputs (column form, N=1 -> f32)
        encw_f = encw if GDT != mybir.dt.float32r else encw.bitcast(f32)
        ps_app = ps.tile([P, 2], f32, name="ps_app", tag="tp", bufs=1)
        for hc in range(2):
            for jc in range(2):
                nc.tensor.matmul(ps_app[:, hc:hc + 1],
                                 lhsT=encw_f[:, jc, hc * P:(hc + 1) * P],
                                 rhs=awc_sb[:, jc:jc + 1],
                                 start=(jc == 0), stop=(jc == 1))
        app_sb = sm.tile([P, 2], GDT, name="app_sb")
        nc.vector.tensor_copy(out=app_sb[:, :], in_=ps_app[:, :])

        # x = relu(comb_in @ comb_w.T + comb_b)  (row form; relu on DVE)
        ps_x = ps.tile([1, HIDDEN], f32, name="ps_x", tag="in", bufs=2)
        nc.tensor.matmul(ps_x[0:1, :], lhsT=one, rhs=pb(_OFF_COMB_B, 256), start=True, stop=False)
        for kc in (0, 1):
            nc.tensor.matmul(ps_x[0:1, :], lhsT=e_col[:, kc:kc + 1],
                             rhs=combw[:, kc, :], start=False, stop=False)
        for kc in (2, 3):
            nc.tensor.matmul(ps_x[0:1, :], lhsT=app_sb[:, kc - 2:kc - 1],
                             rhs=combw[:, kc, :], start=False, stop=(kc == 3))
        x_row = sm.tile([1, HIDDEN], f32, name="x_row")
        nc.vector.tensor_scalar_max(out=x_row[:, :], in0=ps_x[0:1, :], scalar1=0.0)

        def to_col(row, name, dt_):
            pst = ps.tile([P, 2], f32, name=f"ps_{name}", tag="tp", bufs=1)
            nc.tensor.transpose(out=pst[:, 0:1], in_=row[0:1, 0:P], identity=ones_col[0:1, 0:1])
            nc.tensor.transpose(out=pst[:, 1:2], in_=row[0:1, P:2 * P], identity=ones_col[0:1, 0:1])
            col = sm.tile([P, 2], dt_, name=name, tag="xcol", bufs=3)
            nc.vector.tensor_copy(out=col[:, :], in_=pst[:, :])
            return col

        x_col = to_col(x_row, "x_col0", GDT)

        # ---------------- GRU: hidden-side products + biases, accumulated --------
        # directly into each layer's psum groups, off the critical path.
        ps_rz_l, ps_hn_l, ps_in_l = [], [], []
        for l in range(NLAYERS):
            ps_rz = ps.tile([1, 512], f32, name=f"ps_rz{l}", tag="rz", bufs=2)
            ps_hn = ps.tile([1, 256], f32, name=f"ps_hn{l}", tag="hn", bufs=1)
            ps_in = ps.tile([1, 256], f32, name=f"ps_in{l}", tag="in", bufs=2)
            ob = _OFF_BIAS_GH + l * 768
            nc.tensor.matmul(ps_rz[0:1, :], lhsT=one, rhs=pb(ob, 512), start=True, stop=False)
            for kc in range(2):
                nc.tensor.matmul(ps_rz[0:1, :], lhsT=hid_col[:, l, kc:kc + 1],
                                 rhs=gw[l][:, 1, kc, 0:512], start=False, stop=False)
            nc.tensor.matmul(ps_hn[0:1, :], lhsT=one, rhs=pb(ob + 512, 256), start=True, stop=False)
            for kc in range(2):
                nc.tensor.matmul(ps_hn[0:1, :], lhsT=hid_col[:, l, kc:kc + 1],
                                 rhs=gw[l][:, 1, kc, 512:768], start=False, stop=(kc == 1))
            nc.tensor.matmul(ps_in[0:1, :], lhsT=one, rhs=pb(_OFF_BIN + l * 256, 256),
                             start=True, stop=False)
            ps_rz_l.append(ps_rz); ps_hn_l.append(ps_hn); ps_in_l.append(ps_in)

        # ---------------- GRU chain ----------------------------------------------
        for l in range(NLAYERS):
            ps_rz, ps_hn, ps_in = ps_rz_l[l], ps_hn_l[l], ps_in_l[l]
            for kc in range(2):
                nc.tensor.matmul(ps_rz[0:1, :], lhsT=x_col[:, kc:kc + 1],
                                 rhs=gw[l][:, 0, kc, 0:512], start=False, stop=(kc == 1))
                nc.tensor.matmul(ps_in[0:1, :], lhsT=x_col[:, kc:kc + 1],
                                 rhs=gw[l][:, 0, kc, 512:768], start=False, stop=(kc == 1))

            rz = sm.tile([1, 512], f32, name="rz", tag="rz_sb", bufs=1)
            nc.scalar.activation(out=rz[:, :], in_=ps_rz[0:1, :], func=AF.Sigmoid)
            rhn = sm.tile([1, 256], f32, name="rhn", tag="gtmp", bufs=3)
            nc.vector.tensor_mul(out=rhn[:, :], in0=rz[0:1, 0:256], in1=ps_hn[0:1, :])
            npre = sm.tile([1, 256], f32, name="npre", tag="gtmp", bufs=3)
            nc.vector.tensor_add(out=npre[:, :], in0=ps_in[0:1, :], in1=rhn[:, :])
            # tanh(v) = 2*sigmoid(2v) - 1  (keeps the sigmoid ACT table resident)
            sg2 = sm.tile([1, 256], f32, name="sg2", tag="gtmp", bufs=3)
            nc.scalar.activation(out=sg2[:, :], in_=npre[:, :], func=AF.Sigmoid, scale=2.0)
            n_sb = sm.tile([1, 256], f32, name="n_sb", tag="gtmp", bufs=3)
            nc.vector.tensor_scalar(out=n_sb[:, :], in0=sg2[:, :], scalar1=2.0, scalar2=1.0,
                                    op0=OP.mult, op1=OP.subtract)
            d_sb = sm.tile([1, 256], f32, name="d_sb", tag="gtmp", bufs=3)
            nc.vector.tensor_sub(out=d_sb[:, :], in0=pb(_OFF_HID_ROW + l * 256, 256),
                                 in1=n_sb[:, :])
            zd_sb = sm.tile([1, 256], f32, name="zd_sb", tag="gtmp", bufs=3)
            nc.vector.tensor_mul(out=zd_sb[:, :], in0=rz[0:1, 256:512], in1=d_sb[:, :])
            h_row = sm.tile([1, 256], f32, name="h_row", tag="h_row", bufs=2)
            nc.vector.tensor_add(out=h_row[:, :], in0=n_sb[:, :], in1=zd_sb[:, :])
            nc.scalar.dma_start(out=hidout_d.ap()[l:l + 1, :], in_=h_row[:, :])
            x_col = to_col(h_row, f"x_col{l + 1}", WDT if l == NLAYERS - 1 else GDT)

        # ---------------- logits = x @ out_w.T + out_b  (vocab shard) ------------
        lg_sb = sm.tile([1, VP], f32, name="lg_sb")
        sacc = sm.tile([1, 8], f32, name="sacc")
        xv = x_col[:, :]
        for i, (v0, n) in enumerate(_VT):
            ps_row = ps.tile([1, 512], f32, name=f"ps_row{i}", tag="row", bufs=2)
            nc.tensor.matmul(ps_row[0:1, 0:n], lhsT=xv[:, 0:1], rhs=wtv(0, v0, n),
                             start=True, stop=False)
            nc.tensor.matmul(ps_row[0:1, 0:n], lhsT=xv[:, 1:2], rhs=wtv(1, v0, n),
                             start=False, stop=True)
            nc.vector.tensor_add(out=lg_sb[0:1, v0:v0 + n], in0=ps_row[0:1, 0:n],
                                 in1=pb(_OFF_OUTB + v0, n))

        # sum(exp(logits)) in 4 coarse slices on ACT
        dummy = sm.tile([1, 1600], f32, name="dummy", tag="dummy", bufs=2)
        for s in range(4):
            nc.scalar.activation(out=dummy[0:1, :], in_=lg_sb[0:1, s * 1600:(s + 1) * 1600],
                                 func=AF.Exp, accum_out=sacc[0:1, s:s + 1])

        s_sb = sm.tile([1, 8], f32, name="s_sb")
        nc.vector.memset(s_sb, 0.0)
        nc.vector.tensor_reduce(out=s_sb[0:1, 0:1], in_=sacc[0:1, 0:4],
                                axis=mybir.AxisListType.X, op=OP.add)

        # ---------------- 8-way AllGather of the partial sums ---------------------
        cc_in = dr.tile([1, 8], f32, name="cc_in")
        cc_out = dr.tile([NCORES, 8], f32, name="cc_out", addr_space="Shared")
        nc.gpsimd.dma_start(out=cc_in[:, :], in_=s_sb[:, :])
        nc.gpsimd.collective_compute(
            "AllGather", OP.bypass,
            replica_groups=[list(range(NCORES))],
            ins=[cc_in.opt()], outs=[cc_out.opt()],
        )
        sg_sb = sm.tile([NCORES, 8], f32, name="sg_sb")
        nc.gpsimd.dma_start(out=sg_sb[:, :], in_=cc_out[:, :])

        ps_sg = ps.tile([1, 1], f32, name="ps_sg", tag="tp", bufs=1)
        nc.tensor.matmul(ps_sg[0:1, 0:1], lhsT=sg_sb[:, 0:1], rhs=ones_col[0:NCORES, 0:1],
                         start=True, stop=True)
        ln_sb = sm.tile([1, 1], f32, name="ln_sb")
        nc.scalar.activation(out=ln_sb[:, :], in_=ps_sg[0:1, 0:1], func=AF.Ln)
        negln = sm.tile([1, 1], f32, name="negln")
        nc.vector.tensor_scalar_mul(out=negln[:, :], in0=ln_sb[:, :], scalar1=-1.0)

        # logp = logits - log(sum_exp), in place; alternate DVE/ACT slices
        for i in range(8):
            v0, n = i * 800, 800
            if i % 2 == 0:
                nc.vector.tensor_scalar_sub(out=lg_sb[0:1, v0:v0 + n],
                                            in0=lg_sb[0:1, v0:v0 + n],
                                            scalar1=ln_sb[0:1, 0:1])
            else:
                nc.scalar.activation(out=lg_sb[0:1, v0:v0 + n],
                                     in_=lg_sb[0:1, v0:v0 + n],
                                     func=AF.Identity, bias=negln[0:1, 0:1])
        nc.sync.dma_start(out=logp_d.ap(), in_=lg_sb[:, :])

    nc.compile()
    return nc


def _get_nc():
    key = (WDT_NAME, GDT_NAME)
    if key not in _CACHE:
        _CACHE[key] = _build(*key)
    return _CACHE[key]


def _np_dt(name):
    if name == "bf16":
        import ml_dtypes
        return np.dtype(ml_dtypes.bfloat16)
    return np.float32


def _prep_in_maps(inputs):
    f = lambda k: np.ascontiguousarray(np.asarray(inputs[k], dtype=np.float32))
    gnp = _np_dt(GDT_NAME)
    wnp = _np_dt(WDT_NAME)

    token = np.asarray(inputs["token"]).astype(np.int32).reshape(1, 1)
    emb = f("emb")

    hidden = f("hidden").reshape(NLAYERS, HIDDEN)
    hid_colmat = hidden.reshape(NLAYERS, 2, P).transpose(2, 0, 1).reshape(P, NLAYERS * 2)

    pack1 = np.concatenate([
        f("attn_w").T.reshape(4, P, MAXLEN).transpose(1, 0, 2).reshape(P, 1024),
        f("encoder_outputs").reshape(2, P, HIDDEN).transpose(1, 0, 2).reshape(P, 512),
        hid_colmat,
    ], axis=1).astype(gnp)
    pack2 = f("comb_w").T.reshape(4, P, HIDDEN).transpose(1, 0, 2).reshape(P, 1024).astype(gnp)
    assert pack1.shape == (P, 1544)

    bih, bhh = f("gru_bih"), f("gru_bhh")
    b_pad = np.full((VPAD,), NEG, np.float32)
    b_pad[:VOCAB] = f("out_b")
    packb = np.zeros((1, _PACKB_LEN), np.float32)
    packb[0, _OFF_ATTN_B:_OFF_ATTN_B + 256] = f("attn_b")
    packb[0, _OFF_COMB_B:_OFF_COMB_B + 256] = f("comb_b")
    for l in range(NLAYERS):
        o = _OFF_BIAS_GH + l * 768
        packb[0, o:o + 512] = bih[l, :512] + bhh[l, :512]   # brz (merged)
        packb[0, o + 512:o + 768] = bhh[l, 512:]            # bhn
        packb[0, _OFF_BIN + l * 256:_OFF_BIN + (l + 1) * 256] = bih[l, 512:]
        packb[0, _OFF_HID_ROW + l * 256:_OFF_HID_ROW + (l + 1) * 256] = hidden[l]

    wih, whh = f("gru_wih"), f("gru_whh")
    gru_wt = np.ascontiguousarray(
        np.stack([
            np.stack([wih[l].T.reshape(2, P, 768), whh[l].T.reshape(2, P, 768)])
            for l in range(NLAYERS)
        ])
    ).astype(gnp)  # [4, 2, 2, 128, 768]

    w_pad = np.zeros((VPAD, HIDDEN), np.float32)
    w_pad[:VOCAB] = f("out_w")

    common = dict(token=token, emb=emb, pack1=pack1, pack2=pack2, packb=packb, gru_wt=gru_wt)
    in_maps = []
    for k in range(NCORES):
        rows = slice(k * VP, (k + 1) * VP)
        wt = np.ascontiguousarray(w_pad[rows].T.reshape(2, P, VP)).astype(wnp)
        pb = packb.copy()
        pb[0, _OFF_OUTB:] = b_pad[rows]
        in_maps.append(dict(common, wt=wt, packb=pb))
    return in_maps


def _ensure_ntff_hook():
    """The agent image's ``antenv`` lacks ``axon_hooks``; shim it so
    run_bass_kernel_spmd(trace=True) can capture NTFF profiles via the
    libaxon_pjrt.so C ABI (same mechanism as trn_agent_boot)."""
    import sys
    import types
    import contextlib
    import ctypes

    try:
        from antenv.axon_hooks import get_axon_ntff_profile_hook  # noqa: F401
        return
    except ImportError:
        pass

    mod = types.ModuleType("antenv.axon_hooks")
    _state = {"hook": None}
    mod.set_axon_ntff_profile_hook = lambda h: _state.__setitem__("hook", h)
    mod.get_axon_ntff_profile_hook = lambda: _state["hook"]
    sys.modules["antenv.axon_hooks"] = mod

    so_path = "/opt/axon/libaxon_pjrt.so"
    try:
        lib = ctypes.CDLL(so_path)
    except OSError:
        return
    if not hasattr(lib, "axon_start_nrt_profile"):
        return
    lib.axon_start_nrt_profile.argtypes = [ctypes.POINTER(ctypes.c_int64), ctypes.c_size_t]
    lib.axon_start_nrt_profile.restype = ctypes.c_int64
    lib.axon_stop_nrt_profile.argtypes = [ctypes.c_char_p]
    lib.axon_stop_nrt_profile.restype = ctypes.c_int64

    @contextlib.contextmanager
    def _hook(output_dir, device_ids):
        import jax

        jax.devices()
        if device_ids:
            ids = (ctypes.c_int64 * len(device_ids))(*device_ids)
            rc = lib.axon_start_nrt_profile(ids, len(device_ids))
        else:
            rc = lib.axon_start_nrt_profile(None, 0)
        if rc != 0:
            raise RuntimeError(f"axon_start_nrt_profile rc={rc}")
        try:
            yield
        finally:
            n = lib.axon_stop_nrt_profile(str(output_dir).encode())
            print(f"ntff profile: {n} file(s) written to {output_dir}")

    mod.set_axon_ntff_profile_hook(_hook)


def run(inputs, trace=False, trace_cores=None):
    from concourse.bass_utils import run_bass_kernel_spmd

    if trace:
        _ensure_ntff_hook()
    nc = _get_nc()
    in_maps = _prep_in_maps(inputs)
    res = run_bass_kernel_spmd(nc, in_maps, core_ids=list(range(NCORES)), trace=trace,
                               trace_cores=trace_cores)
    outs = res.results
    logp = np.concatenate(
        [outs[k]["logp"][0] for k in range(NCORES)]
    )[:VOCAB].reshape(1, VOCAB).astype(np.float32)
    hidden_new = np.ascontiguousarray(outs[0]["hidden_out"].reshape(NLAYERS, 1, HIDDEN))
    attn_weights = np.ascontiguousarray(outs[0]["attn_out"].reshape(1, MAXLEN))
    return (logp, hidden_new, attn_weights), res


def kernel(**inputs):
    (logp, hidden_new, attn_weights), _ = run(inputs, trace=False)
    return logp, hidden_new, attn_weights
